# revision 18
# baseline (speedup 1.0000x reference)
"""Trainium2 Bass kernel: 2D positional embeddings with bilinear interpolation.

Problem: positions (8,256,256,2) f32 in [0,1)^2, embeddings (64,64,128) f32.
out[..., :] = bilinear interp of the 64x64x128 grid at each query point.

Strategy (8 NeuronCores, data-parallel over the 524288 query points):
  - Host: rewrite the embedding table into a per-cell finite-difference
    basis  T[cell] = [m, bx, by, bxy]  (cell = x0*64 + y0), so that
        out = m + dx*bx + dy*by + (dx*dy)*bxy
    is exactly the bilinear interpolation (y/x clipping folded into T).
  - Device (per core, 65536 points):
      * compute cell indices (int16, SWDGE-wrapped layout) and the
        fractional offsets dx/dy (slot-major [128, T]) on the DVE,
      * dma_gather the 4*128-float basis entry for every point from HBM
        into point-major SBUF tiles [128 points, 512],
      * combine with 3 fused scalar_tensor_tensor ops per 128-point tile
        (per-partition scalars dx, dy),
      * stream results back to HBM as contiguous 512B rows per point.
  - Host: concatenate the 8 per-core outputs.

The positions are staged host-side in the two layouts the device wants
(slot-major for dx/dy, wrapped-replicated for the gather indices) so every
DMA load is contiguous per partition.
"""

import numpy as np

import concourse.bass as bass
import concourse.tile as tile
from concourse import bacc, mybir
from concourse.mybir import AluOpType as alu

GRID = 64
F = 128
N_CORES = 8
NPC = 8 * 256 * 256 // N_CORES  # points per core = 65536

N_QUEUES = 2  # alternate SWDGE queues so desc-gen overlaps drain
CH = 1024  # points per gather chunk (SWDGE desc-ring limit: keep <= 1024)
WCH = 512  # wrapped-compute chunk (slots)

# Table entry: [m f32 (256 u16) | bx f16 (128) | by f16 (128) | bxy f16 (128)]
# = 640 f16-units = 1280 B. m exact in f32; the difference terms are small
# (grid-neighbor deltas, |.| < ~10) so fp16 there costs < 0.1% worst-case.
EU = 640  # entry size in f16 units

f32 = mybir.dt.float32
f16 = mybir.dt.float16
i16 = mybir.dt.int16


def build_basis_table(emb: np.ndarray) -> np.ndarray:
    """emb [64, 64, 128] f32 -> packed [4096, 640] uint16 mixed-precision table."""
    G = GRID
    x0 = np.arange(G)
    x1 = np.minimum(x0 + 1, G - 1)
    e00 = emb[x0][:, x0].reshape(G * G, F)
    e10 = emb[x1][:, x0].reshape(G * G, F)
    e01 = emb[x0][:, x1].reshape(G * G, F)
    e11 = emb[x1][:, x1].reshape(G * G, F)
    m = e00.astype(np.float32)
    bx = (e10 - e00).astype(np.float16)
    by = (e01 - e00).astype(np.float16)
    bxy = (e11 - e10 - (e01 - e00)).astype(np.float16)
    return np.ascontiguousarray(
        np.concatenate(
            [m.view(np.uint16).reshape(G * G, 2 * F), bx.view(np.uint16),
             by.view(np.uint16), bxy.view(np.uint16)],
            axis=1,
        )
    )


def build_nc(npc: int = NPC, ch: int = CH, wch: int = WCH):
    """Build the single-core Bass program (same NEFF runs on all 8 cores)."""
    import ml_dtypes  # noqa: F401

    S = npc // 16  # wrapped idx slots
    T = npc // 128  # slot-major tiles
    TC = ch // 128  # tiles per chunk
    n_chunks = npc // ch
    n_wchunks = S // wch

    nc = bacc.Bacc(
        "TRN2",
        target_bir_lowering=False,
        debug=False,
        num_devices=N_CORES,
        num_swdge_queues=N_QUEUES,
    )

    pos_pm = nc.dram_tensor("pos_pm", [128, T, 2], f32, kind="ExternalInput").ap()
    pos_w = nc.dram_tensor("pos_w", [128, S, 2], f32, kind="ExternalInput").ap()
    table = nc.dram_tensor("table", [GRID * GRID, EU], f16, kind="ExternalInput").ap()
    out = nc.dram_tensor("out", [npc, F], f32, kind="ExternalOutput").ap()
    out_r = out.rearrange("(t p) f -> p t f", p=128)

    with tile.TileContext(nc) as tc:
        with (
            tc.tile_pool(name="const", bufs=1) as const,
            tc.tile_pool(name="wscr", bufs=2) as wscr,
            tc.tile_pool(name="gbuf", bufs=2) as gpool,
            tc.tile_pool(name="obuf", bufs=2) as opool,
            tc.tile_pool(name="tbuf", bufs=4) as tpool,
        ):
            def emit_floor(pool, xs, n):
                """floor(xs) for xs >= 0, robust to HW int-cast rounding mode."""
                xi = pool.tile([128, n], mybir.dt.int32, tag="fl_i")
                xf = pool.tile([128, n], f32, tag="fl_f")
                co = pool.tile([128, n], f32, tag="fl_c")
                nc.vector.tensor_copy(xi[:], xs[:])
                nc.vector.tensor_copy(xf[:], xi[:])
                nc.vector.tensor_tensor(co[:], xf[:], xs[:], alu.is_gt)
                nc.vector.tensor_sub(xf[:], xf[:], co[:])
                return xf

            # ---- slot-major fractional offsets dx, dy [128, T] f32 ----
            pm = const.tile([128, T, 2], f32)
            nc.sync.dma_start(pm[:], pos_pm)
            dx = const.tile([128, T], f32)
            dy = const.tile([128, T], f32)
            nc.vector.tensor_scalar_mul(dx[:], pm[:, :, 0], float(GRID))
            x0 = emit_floor(wscr, dx, T)
            nc.vector.tensor_sub(dx[:], dx[:], x0[:])
            nc.vector.tensor_scalar_mul(dy[:], pm[:, :, 1], float(GRID))
            y0 = emit_floor(wscr, dy, T)
            nc.vector.tensor_sub(dy[:], dy[:], y0[:])

            # ---- wrapped cell indices [128, S] int16 ----
            cell16 = const.tile([128, S], i16)
            for w in range(n_wchunks):
                pw = wscr.tile([128, wch, 2], f32)
                nc.sync.dma_start(pw[:], pos_w[:, bass.ts(w, wch), :])
                xs = wscr.tile([128, wch], f32)
                ys = wscr.tile([128, wch], f32)
                nc.vector.tensor_scalar_mul(xs[:], pw[:, :, 0], float(GRID))
                wx0 = emit_floor(wscr, xs, wch)
                nc.vector.tensor_scalar_mul(ys[:], pw[:, :, 1], float(GRID))
                wy0 = emit_floor(wscr, ys, wch)
                nc.vector.scalar_tensor_tensor(
                    xs[:], wx0[:], float(GRID), wy0[:], op0=alu.mult, op1=alu.add
                )
                nc.vector.tensor_copy(cell16[:, bass.ts(w, wch)], xs[:])

            # ---- main loop: gather + combine + store ----
            # out = (m + dx*bx) + dy*(by + dx*bxy)
            # ACT does the per-tile dx-multiplies (per-partition scale is its
            # native form); DVE does two chunk-wide adds + one fused
            # mult-add (stt) per tile for the dy term.
            for c in range(n_chunks):
                g = gpool.tile([128, TC, EU], f16)
                idxs = cell16[:, bass.ds(c * ch // 16, ch // 16)]
                nc.gpsimd.dma_gather(
                    g[:], table, idxs, ch, ch, EU, queue_num=c % N_QUEUES
                )
                g_m = g[:].bitcast(f32)[:, :, 0:F]  # [128, TC, F] f32
                g_bx = g[:, :, 2 * F : 3 * F]
                g_by = g[:, :, 3 * F : 4 * F]
                g_bxy = g[:, :, 4 * F : 5 * F]

                a1 = tpool.tile([128, TC, F], f32, tag="a1")
                a2 = tpool.tile([128, TC, F], f32, tag="a2")
                for t in range(TC):
                    dxs = dx[:, bass.ds(c * TC + t, 1)]
                    nc.scalar.mul(a1[:, t], g_bx[:, t], dxs)  # dx*bx
                    nc.scalar.mul(a2[:, t], g_bxy[:, t], dxs)  # dx*bxy
                s1 = tpool.tile([128, TC, F], f32, tag="s1")
                s2 = tpool.tile([128, TC, F], f32, tag="s2")
                nc.vector.tensor_add(s1[:], g_m, a1[:])  # m + dx*bx
                nc.vector.tensor_add(s2[:], g_by, a2[:])  # by + dx*bxy

                ob = opool.tile([128, TC, F], f32)
                for t in range(TC):
                    dys = dy[:, bass.ds(c * TC + t, 1)]
                    nc.vector.scalar_tensor_tensor(
                        ob[:, t], s2[:, t], dys, s1[:, t],
                        op0=alu.mult, op1=alu.add,
                    )
                nc.sync.dma_start(out_r[:, bass.ds(c * TC, TC), :], ob[:])

    nc.compile()
    return nc


def stage_inputs(positions: np.ndarray, embeddings: np.ndarray):
    """Split FULL inputs into per-core in_maps for the device program."""
    table = build_basis_table(np.asarray(embeddings, dtype=np.float32)).view(
        np.float16
    )
    pos_flat = np.asarray(positions, dtype=np.float32).reshape(-1, 2)
    in_maps = []
    for c in range(N_CORES):
        p = pos_flat[c * NPC : (c + 1) * NPC]
        # slot-major: slot (part, t) <-> point t*128 + part
        pos_pm = np.ascontiguousarray(p.reshape(NPC // 128, 128, 2).transpose(1, 0, 2))
        # wrapped: slot s*16+q -> (q, s); replicated across the 8 Q7 core groups
        pw = p.reshape(NPC // 16, 16, 2).transpose(1, 0, 2)
        pos_w = np.ascontiguousarray(np.tile(pw, (8, 1, 1)))
        in_maps.append({"pos_pm": pos_pm, "pos_w": pos_w, "table": table})
    return in_maps


_NC_CACHE = {}


def kernel(positions: np.ndarray, embeddings: np.ndarray) -> np.ndarray:
    from concourse.bass_utils import run_bass_kernel_spmd

    if "nc" not in _NC_CACHE:
        _NC_CACHE["nc"] = build_nc()
    nc = _NC_CACHE["nc"]
    in_maps = stage_inputs(positions, embeddings)
    res = run_bass_kernel_spmd(nc, in_maps, core_ids=list(range(N_CORES)))
    full = np.concatenate([res.results[c]["out"] for c in range(N_CORES)], axis=0)
    return full.reshape(8, 256, 256, F).astype(np.float32)


# revision 22
# speedup vs baseline: 1.6404x; 1.6404x over previous
"""Trainium2 Bass kernel: 2D positional embeddings with bilinear interpolation.

Problem: positions (8,256,256,2) f32 in [0,1)^2, embeddings (64,64,128) f32.
out[..., :] = bilinear interp of the 64x64x128 grid at each query point.

Strategy (8 NeuronCores, data-parallel over the 524288 query points):
  - Host: rewrite the embedding table into a per-cell finite-difference
    basis  T[cell] = [m, bx, by, bxy]  (cell = x0*64 + y0), so that
        out = m + dx*bx + dy*by + (dx*dy)*bxy
    is exactly the bilinear interpolation (y/x clipping folded into T).
  - Device (per core, 65536 points):
      * compute cell indices (int16, SWDGE-wrapped layout) and the
        fractional offsets dx/dy (slot-major [128, T]) on the DVE,
      * dma_gather the 4*128-float basis entry for every point from HBM
        into point-major SBUF tiles [128 points, 512],
      * combine with 3 fused scalar_tensor_tensor ops per 128-point tile
        (per-partition scalars dx, dy),
      * stream results back to HBM as contiguous 512B rows per point.
  - Host: concatenate the 8 per-core outputs.

The positions are staged host-side in the two layouts the device wants
(slot-major for dx/dy, wrapped-replicated for the gather indices) so every
DMA load is contiguous per partition.
"""

import numpy as np

import concourse.bass as bass
import concourse.tile as tile
from concourse import bacc, mybir
from concourse.mybir import AluOpType as alu

GRID = 64
F = 128
N_CORES = 8
NPC = 8 * 256 * 256 // N_CORES  # points per core = 65536

N_QUEUES = 2  # alternate SWDGE queues so desc-gen overlaps drain
CH = 1024  # points per gather chunk (SWDGE desc-ring limit: keep <= 1024)
WCH = 512  # wrapped-compute chunk (slots)

# Table entry: [m | bx | bxy | by], each 128 fp16 = 1024 B total.
# fp16 keeps ~11-bit mantissas: |values| < ~10 here, worst-case error ~0.2%.
# bx and bxy are adjacent so one ACT op applies the dx scale to both.
EU = 512  # entry size in f16 units

f32 = mybir.dt.float32
f16 = mybir.dt.float16
i16 = mybir.dt.int16


def build_basis_table(emb: np.ndarray) -> np.ndarray:
    """emb [64, 64, 128] f32 -> [4096, 512] fp16 basis table [m, bx, bxy, by]."""
    G = GRID
    x0 = np.arange(G)
    x1 = np.minimum(x0 + 1, G - 1)
    e00 = emb[x0][:, x0].reshape(G * G, F)
    e10 = emb[x1][:, x0].reshape(G * G, F)
    e01 = emb[x0][:, x1].reshape(G * G, F)
    e11 = emb[x1][:, x1].reshape(G * G, F)
    m = e00
    bx = e10 - e00
    by = e01 - e00
    bxy = e11 - e10 - (e01 - e00)
    t = np.concatenate([m, bx, bxy, by], axis=-1)
    return np.ascontiguousarray(t.astype(np.float16))


def build_nc(npc: int = NPC, ch: int = CH, wch: int = WCH):
    """Build the single-core Bass program (same NEFF runs on all 8 cores)."""
    import ml_dtypes  # noqa: F401

    S = npc // 16  # wrapped idx slots
    T = npc // 128  # slot-major tiles
    TC = ch // 128  # tiles per chunk
    n_chunks = npc // ch
    n_wchunks = S // wch

    nc = bacc.Bacc(
        "TRN2",
        target_bir_lowering=False,
        debug=False,
        num_devices=N_CORES,
        num_swdge_queues=N_QUEUES,
    )

    pos_pm = nc.dram_tensor("pos_pm", [128, T, 2], f32, kind="ExternalInput").ap()
    pos_w = nc.dram_tensor("pos_w", [128, S, 2], f32, kind="ExternalInput").ap()
    table = nc.dram_tensor("table", [GRID * GRID, EU], f16, kind="ExternalInput").ap()
    out = nc.dram_tensor("out", [npc, F], f32, kind="ExternalOutput").ap()
    out_r = out.rearrange("(t p) f -> p t f", p=128)

    with tile.TileContext(nc) as tc:
        with (
            tc.tile_pool(name="const", bufs=1) as const,
            tc.tile_pool(name="wscr", bufs=2) as wscr,
            tc.tile_pool(name="gbuf", bufs=3) as gpool,
            tc.tile_pool(name="obuf", bufs=2) as opool,
            tc.tile_pool(name="tbuf", bufs=4) as tpool,
        ):
            def emit_floor(pool, xs, n):
                """floor(xs) for xs >= 0, robust to HW int-cast rounding mode."""
                xi = pool.tile([128, n], mybir.dt.int32, tag="fl_i")
                xf = pool.tile([128, n], f32, tag="fl_f")
                co = pool.tile([128, n], f32, tag="fl_c")
                nc.vector.tensor_copy(xi[:], xs[:])
                nc.vector.tensor_copy(xf[:], xi[:])
                nc.vector.tensor_tensor(co[:], xf[:], xs[:], alu.is_gt)
                nc.vector.tensor_sub(xf[:], xf[:], co[:])
                return xf

            # ---- slot-major fractional offsets dx, dy [128, T] f32 ----
            pm = const.tile([128, T, 2], f32)
            nc.sync.dma_start(pm[:], pos_pm)
            dx = const.tile([128, T], f32)
            dy = const.tile([128, T], f32)
            nc.vector.tensor_scalar_mul(dx[:], pm[:, :, 0], float(GRID))
            x0 = emit_floor(wscr, dx, T)
            nc.vector.tensor_sub(dx[:], dx[:], x0[:])
            nc.vector.tensor_scalar_mul(dy[:], pm[:, :, 1], float(GRID))
            y0 = emit_floor(wscr, dy, T)
            nc.vector.tensor_sub(dy[:], dy[:], y0[:])

            # ---- wrapped cell indices [128, S] int16 ----
            cell16 = const.tile([128, S], i16)
            for w in range(n_wchunks):
                pw = wscr.tile([128, wch, 2], f32)
                nc.sync.dma_start(pw[:], pos_w[:, bass.ts(w, wch), :])
                xs = wscr.tile([128, wch], f32)
                ys = wscr.tile([128, wch], f32)
                nc.vector.tensor_scalar_mul(xs[:], pw[:, :, 0], float(GRID))
                wx0 = emit_floor(wscr, xs, wch)
                nc.vector.tensor_scalar_mul(ys[:], pw[:, :, 1], float(GRID))
                wy0 = emit_floor(wscr, ys, wch)
                nc.vector.scalar_tensor_tensor(
                    xs[:], wx0[:], float(GRID), wy0[:], op0=alu.mult, op1=alu.add
                )
                nc.vector.tensor_copy(cell16[:, bass.ts(w, wch)], xs[:])

            # ---- main loop: gather + combine + store ----
            # out = (m + dx*bx) + dy*(by + dx*bxy)
            # ACT does the per-tile dx-multiplies (per-partition scale is its
            # native form); DVE does two chunk-wide adds + one fused
            # mult-add (stt) per tile for the dy term.
            for c in range(n_chunks):
                g = gpool.tile([128, TC, EU], f16)
                idxs = cell16[:, bass.ds(c * ch // 16, ch // 16)]
                nc.gpsimd.dma_gather(
                    g[:], table, idxs, ch, ch, EU, queue_num=c % N_QUEUES
                )
                g_m = g[:, :, 0:F]
                g_bb = g[:, :, F : 3 * F]  # [bx | bxy] pairs
                g_by = g[:, :, 3 * F : 4 * F]

                a12 = tpool.tile([128, TC, 2 * F], f16, tag="a12")
                for t in range(TC):
                    dxs = dx[:, bass.ds(c * TC + t, 1)]
                    nc.scalar.mul(a12[:, t], g_bb[:, t], dxs)  # [dx*bx | dx*bxy]
                s1 = tpool.tile([128, TC, F], f16, tag="s1")
                s2 = tpool.tile([128, TC, F], f16, tag="s2")
                a12v = a12[:].rearrange("p t (k f) -> p t k f", k=2)
                nc.vector.tensor_add(s1[:], g_m, a12v[:, :, 0, :])  # m + dx*bx
                nc.vector.tensor_add(s2[:], g_by, a12v[:, :, 1, :])  # by + dx*bxy

                ob = opool.tile([128, TC, F], f32)
                for t in range(TC):
                    dys = dy[:, bass.ds(c * TC + t, 1)]
                    nc.vector.scalar_tensor_tensor(
                        ob[:, t], s2[:, t], dys, s1[:, t],
                        op0=alu.mult, op1=alu.add,
                    )
                nc.sync.dma_start(out_r[:, bass.ds(c * TC, TC), :], ob[:])

    nc.compile()
    return nc


def stage_inputs(positions: np.ndarray, embeddings: np.ndarray):
    """Split FULL inputs into per-core in_maps for the device program."""
    table = build_basis_table(np.asarray(embeddings, dtype=np.float32))
    pos_flat = np.asarray(positions, dtype=np.float32).reshape(-1, 2)
    in_maps = []
    for c in range(N_CORES):
        p = pos_flat[c * NPC : (c + 1) * NPC]
        # slot-major: slot (part, t) <-> point t*128 + part
        pos_pm = np.ascontiguousarray(p.reshape(NPC // 128, 128, 2).transpose(1, 0, 2))
        # wrapped: slot s*16+q -> (q, s); replicated across the 8 Q7 core groups
        pw = p.reshape(NPC // 16, 16, 2).transpose(1, 0, 2)
        pos_w = np.ascontiguousarray(np.tile(pw, (8, 1, 1)))
        in_maps.append({"pos_pm": pos_pm, "pos_w": pos_w, "table": table})
    return in_maps


_NC_CACHE = {}


def kernel(positions: np.ndarray, embeddings: np.ndarray) -> np.ndarray:
    from concourse.bass_utils import run_bass_kernel_spmd

    if "nc" not in _NC_CACHE:
        _NC_CACHE["nc"] = build_nc()
    nc = _NC_CACHE["nc"]
    in_maps = stage_inputs(positions, embeddings)
    res = run_bass_kernel_spmd(nc, in_maps, core_ids=list(range(N_CORES)))
    full = np.concatenate([res.results[c]["out"] for c in range(N_CORES)], axis=0)
    return full.reshape(8, 256, 256, F).astype(np.float32)


# revision 23
# speedup vs baseline: 1.8958x; 1.1557x over previous
"""Trainium2 Bass kernel: 2D positional embeddings with bilinear interpolation.

Problem: positions (8,256,256,2) f32 in [0,1)^2, embeddings (64,64,128) f32.
out[..., :] = bilinear interp of the 64x64x128 grid at each query point.

Strategy (8 NeuronCores, data-parallel over the 524288 query points):
  - Host: rewrite the embedding table into a per-cell finite-difference
    basis  T[cell] = [m, bx, by, bxy]  (cell = x0*64 + y0), so that
        out = m + dx*bx + dy*by + (dx*dy)*bxy
    is exactly the bilinear interpolation (y/x clipping folded into T).
  - Device (per core, 65536 points):
      * compute cell indices (int16, SWDGE-wrapped layout) and the
        fractional offsets dx/dy (slot-major [128, T]) on the DVE,
      * dma_gather the 4*128-float basis entry for every point from HBM
        into point-major SBUF tiles [128 points, 512],
      * combine with 3 fused scalar_tensor_tensor ops per 128-point tile
        (per-partition scalars dx, dy),
      * stream results back to HBM as contiguous 512B rows per point.
  - Host: concatenate the 8 per-core outputs.

The positions are staged host-side in the two layouts the device wants
(slot-major for dx/dy, wrapped-replicated for the gather indices) so every
DMA load is contiguous per partition.
"""

import numpy as np

import concourse.bass as bass
import concourse.tile as tile
from concourse import bacc, mybir
from concourse.mybir import AluOpType as alu

GRID = 64
F = 128
N_CORES = 8
NPC = 8 * 256 * 256 // N_CORES  # points per core = 65536

N_QUEUES = 4  # alternate SWDGE queues so desc-gen overlaps drain
CH = 1024  # points per gather chunk (SWDGE desc-ring limit: keep <= 1024)
WCH = 512  # wrapped-compute chunk (slots)

# Table entry: [m | bx | bxy | by], each 128 fp16 = 1024 B total.
# fp16 keeps ~11-bit mantissas: |values| < ~10 here, worst-case error ~0.2%.
# bx and bxy are adjacent so one ACT op applies the dx scale to both.
EU = 512  # entry size in f16 units

f32 = mybir.dt.float32
f16 = mybir.dt.float16
i16 = mybir.dt.int16


def build_basis_table(emb: np.ndarray) -> np.ndarray:
    """emb [64, 64, 128] f32 -> [4096, 512] fp16 basis table [m, bx, bxy, by]."""
    G = GRID
    x0 = np.arange(G)
    x1 = np.minimum(x0 + 1, G - 1)
    e00 = emb[x0][:, x0].reshape(G * G, F)
    e10 = emb[x1][:, x0].reshape(G * G, F)
    e01 = emb[x0][:, x1].reshape(G * G, F)
    e11 = emb[x1][:, x1].reshape(G * G, F)
    m = e00
    bx = e10 - e00
    by = e01 - e00
    bxy = e11 - e10 - (e01 - e00)
    t = np.concatenate([m, bx, bxy, by], axis=-1)
    return np.ascontiguousarray(t.astype(np.float16))


def build_nc(npc: int = NPC, ch: int = CH, wch: int = WCH):
    """Build the single-core Bass program (same NEFF runs on all 8 cores)."""
    import ml_dtypes  # noqa: F401

    S = npc // 16  # wrapped idx slots
    T = npc // 128  # slot-major tiles
    TC = ch // 128  # tiles per chunk
    n_chunks = npc // ch
    n_wchunks = S // wch

    nc = bacc.Bacc(
        "TRN2",
        target_bir_lowering=False,
        debug=False,
        num_devices=N_CORES,
        num_swdge_queues=N_QUEUES,
    )

    pos_pm = nc.dram_tensor("pos_pm", [128, T, 2], f32, kind="ExternalInput").ap()
    pos_w = nc.dram_tensor("pos_w", [128, S, 2], f32, kind="ExternalInput").ap()
    table = nc.dram_tensor("table", [GRID * GRID, EU], f16, kind="ExternalInput").ap()
    out = nc.dram_tensor("out", [npc, F], f32, kind="ExternalOutput").ap()
    out_r = out.rearrange("(t p) f -> p t f", p=128)

    with tile.TileContext(nc) as tc:
        with (
            tc.tile_pool(name="const", bufs=1) as const,
            tc.tile_pool(name="wscr", bufs=2) as wscr,
            tc.tile_pool(name="gbuf", bufs=4) as gpool,
            tc.tile_pool(name="obuf", bufs=2) as opool,
            tc.tile_pool(name="tbuf", bufs=4) as tpool,
        ):
            def emit_floor(pool, xs, n):
                """floor(xs) for xs >= 0, robust to HW int-cast rounding mode."""
                xi = pool.tile([128, n], mybir.dt.int32, tag="fl_i")
                xf = pool.tile([128, n], f32, tag="fl_f")
                co = pool.tile([128, n], f32, tag="fl_c")
                nc.vector.tensor_copy(xi[:], xs[:])
                nc.vector.tensor_copy(xf[:], xi[:])
                nc.vector.tensor_tensor(co[:], xf[:], xs[:], alu.is_gt)
                nc.vector.tensor_sub(xf[:], xf[:], co[:])
                return xf

            # ---- slot-major fractional offsets dx, dy [128, T] f32 ----
            pm = const.tile([128, T, 2], f32)
            nc.sync.dma_start(pm[:], pos_pm)
            dx = const.tile([128, T], f32)
            dy = const.tile([128, T], f32)
            nc.vector.tensor_scalar_mul(dx[:], pm[:, :, 0], float(GRID))
            x0 = emit_floor(wscr, dx, T)
            nc.vector.tensor_sub(dx[:], dx[:], x0[:])
            nc.vector.tensor_scalar_mul(dy[:], pm[:, :, 1], float(GRID))
            y0 = emit_floor(wscr, dy, T)
            nc.vector.tensor_sub(dy[:], dy[:], y0[:])

            # ---- wrapped cell indices [128, S] int16 ----
            cell16 = const.tile([128, S], i16)
            for w in range(n_wchunks):
                pw = wscr.tile([128, wch, 2], f32)
                nc.sync.dma_start(pw[:], pos_w[:, bass.ts(w, wch), :])
                xs = wscr.tile([128, wch], f32)
                ys = wscr.tile([128, wch], f32)
                nc.vector.tensor_scalar_mul(xs[:], pw[:, :, 0], float(GRID))
                wx0 = emit_floor(wscr, xs, wch)
                nc.vector.tensor_scalar_mul(ys[:], pw[:, :, 1], float(GRID))
                wy0 = emit_floor(wscr, ys, wch)
                nc.vector.scalar_tensor_tensor(
                    xs[:], wx0[:], float(GRID), wy0[:], op0=alu.mult, op1=alu.add
                )
                nc.vector.tensor_copy(cell16[:, bass.ts(w, wch)], xs[:])

            # ---- main loop: gather + combine + store ----
            # out = (m + dx*bx) + dy*(by + dx*bxy)
            # ACT does the per-tile dx-multiplies (per-partition scale is its
            # native form); DVE does two chunk-wide adds + one fused
            # mult-add (stt) per tile for the dy term.
            for c in range(n_chunks):
                g = gpool.tile([128, TC, EU], f16)
                idxs = cell16[:, bass.ds(c * ch // 16, ch // 16)]
                nc.gpsimd.dma_gather(
                    g[:], table, idxs, ch, ch, EU, queue_num=c % N_QUEUES
                )
                g_m = g[:, :, 0:F]
                g_bb = g[:, :, F : 3 * F]  # [bx | bxy] pairs
                g_by = g[:, :, 3 * F : 4 * F]

                a12 = tpool.tile([128, TC, 2 * F], f16, tag="a12")
                for t in range(TC):
                    dxs = dx[:, bass.ds(c * TC + t, 1)]
                    nc.scalar.mul(a12[:, t], g_bb[:, t], dxs)  # [dx*bx | dx*bxy]
                s1 = tpool.tile([128, TC, F], f16, tag="s1")
                s2 = tpool.tile([128, TC, F], f16, tag="s2")
                a12v = a12[:].rearrange("p t (k f) -> p t k f", k=2)
                nc.vector.tensor_add(s1[:], g_m, a12v[:, :, 0, :])  # m + dx*bx
                nc.vector.tensor_add(s2[:], g_by, a12v[:, :, 1, :])  # by + dx*bxy

                ob = opool.tile([128, TC, F], f32)
                for t in range(TC):
                    dys = dy[:, bass.ds(c * TC + t, 1)]
                    nc.vector.scalar_tensor_tensor(
                        ob[:, t], s2[:, t], dys, s1[:, t],
                        op0=alu.mult, op1=alu.add,
                    )
                nc.sync.dma_start(out_r[:, bass.ds(c * TC, TC), :], ob[:])

    nc.compile()
    return nc


def stage_inputs(positions: np.ndarray, embeddings: np.ndarray):
    """Split FULL inputs into per-core in_maps for the device program."""
    table = build_basis_table(np.asarray(embeddings, dtype=np.float32))
    pos_flat = np.asarray(positions, dtype=np.float32).reshape(-1, 2)
    in_maps = []
    for c in range(N_CORES):
        p = pos_flat[c * NPC : (c + 1) * NPC]
        # slot-major: slot (part, t) <-> point t*128 + part
        pos_pm = np.ascontiguousarray(p.reshape(NPC // 128, 128, 2).transpose(1, 0, 2))
        # wrapped: slot s*16+q -> (q, s); replicated across the 8 Q7 core groups
        pw = p.reshape(NPC // 16, 16, 2).transpose(1, 0, 2)
        pos_w = np.ascontiguousarray(np.tile(pw, (8, 1, 1)))
        in_maps.append({"pos_pm": pos_pm, "pos_w": pos_w, "table": table})
    return in_maps


_NC_CACHE = {}


def kernel(positions: np.ndarray, embeddings: np.ndarray) -> np.ndarray:
    from concourse.bass_utils import run_bass_kernel_spmd

    if "nc" not in _NC_CACHE:
        _NC_CACHE["nc"] = build_nc()
    nc = _NC_CACHE["nc"]
    in_maps = stage_inputs(positions, embeddings)
    res = run_bass_kernel_spmd(nc, in_maps, core_ids=list(range(N_CORES)))
    full = np.concatenate([res.results[c]["out"] for c in range(N_CORES)], axis=0)
    return full.reshape(8, 256, 256, F).astype(np.float32)
